# revision 22
# baseline (speedup 1.0000x reference)
"""GQA attention kernel for Trainium2 (Bass/Tile), 8-core SPMD.  v2.

Problem: B=2, N=2048, DIM=1024, 16 query heads / 4 KV heads, head_dim=64, fp32.
Sharding: core c = (batch b=c//4, kv-group g=c%4).  Each core computes its
group's 4 query heads + 1 shared KV head over the full sequence, and a partial
output projection (its 256 rows of Wo).  Host sums the 4 group partials per
batch and adds the bias.

v2 design (vs baseline):
 - All data bf16 (host-converted); x^T loaded via DMA-transpose (no PE
   transposes, no transpose copies).
 - Scores via fp8 DoubleRow with full error compensation: K^T is stored as
   [K8; K-K8] stacked on 128 partitions (dual e4m3), Q^T as (Q8, Q-Q8) in the
   two DoubleRow slots with rows duplicated.  One DR matmul computes
   (K8+l8)^T (Q8+r8) = exact-ish scores at 0.5 cycles/row (2x fewer PE cycles).
 - V projected directly in [keys, dim] layout (x^T stationary) - no V transpose.
 - exp split between Act engine (exact exp) and DVE (Schraudolph int16->bf16
   bit trick); P@V stays bf16 with the ones-row sum-of-exp trick.
 - Output projection from bf16 aout; host sums partials.
"""

import sys

if "/opt/trn_rl_repo" not in sys.path:
    sys.path.insert(0, "/opt/trn_rl_repo")

from contextlib import ExitStack

import numpy as np

import concourse.bass as bass
import concourse.mybir as mybir
import concourse.tile as tile
from concourse import bacc, bass_utils
from concourse.bass import ds, ts

F32 = mybir.dt.float32
BF16 = mybir.dt.bfloat16
E4M3 = mybir.dt.float8e4
I16 = mybir.dt.int16
EXPF = mybir.ActivationFunctionType.Exp
DR = mybir.MatmulPerfMode.DoubleRow
MULT = mybir.AluOpType.mult
ADD = mybir.AluOpType.add

DIM = 1024
D = 64  # head dim
SCALE = D ** -0.5
LOG2E = float(np.log2(np.e))
A_SCH = 128.0 * LOG2E * SCALE     # schraudolph multiplier on raw scores
C_SCH = 127.0 * 128.0 - 5.57      # schraudolph bias (bf16 bits)


def build_nc(NSEQ=2048):
    KT = NSEQ // 128   # key tiles
    QC = NSEQ // 512   # query chunks of 512
    DKT = DIM // 128   # contraction tiles for projections

    nc = bacc.Bacc("TRN2", target_bir_lowering=False, debug=False)
    xt = nc.dram_tensor("xt", [DIM, NSEQ], BF16, kind="ExternalInput").ap()
    wq = nc.dram_tensor("wq", [DIM, 256], BF16, kind="ExternalInput").ap()
    wkv = nc.dram_tensor("wkv", [DIM, 2 * D], BF16, kind="ExternalInput").ap()
    wo = nc.dram_tensor("wo", [256, DIM], BF16, kind="ExternalInput").ap()
    out = nc.dram_tensor("out", [DIM, NSEQ], BF16, kind="ExternalOutput").ap()
    scr = nc.dram_tensor("scr", [QC, 4, 512], F32, kind="Internal").ap()

    with tile.TileContext(nc) as tc, ExitStack() as ctx:
        sb = ctx.enter_context(tc.tile_pool(name="sb", bufs=1))

        wq_sb = sb.tile([128, DKT, 256], BF16)
        wkv_sb = sb.tile([128, DKT, 2 * D], BF16)
        wo_sb = sb.tile([128, 2, DIM], BF16)
        warm = sb.tile([128, 1], F32)
        ones1 = sb.tile([128, 1], F32)

        xT = sb.tile([128, DKT, NSEQ], BF16)     # x^T: [dim-part, d-tile, seq]

        def load_xt(sg, halves=(0, 1)):
            # host sends x pre-transposed; two plain DMAs per seq chunk,
            # split by d-tiles so the first projection matmuls start early
            for half in halves:
                dst = xT[:, ds(half * 4, 4), ds(sg * 512, 512)]
                src_ = bass.AP(tensor=xt.tensor,
                               offset=xt.offset + half * 4 * 128 * NSEQ + sg * 512,
                               ap=[[NSEQ, 128], [128 * NSEQ, 4], [1, 512]])
                nc.sync.dma_start(out=dst, in_=src_)

        # sg0 x load and wkv interleave at the head of the SP queue so the
        # first K-proj matmuls can start as early as possible.
        load_xt(0, halves=(0,))
        nc.sync.dma_start(out=wq_sb, in_=wq.rearrange("(t p) m -> p t m", p=128))
        load_xt(0, halves=(1,))
        nc.sync.dma_start(out=wkv_sb, in_=wkv.rearrange("(t p) m -> p t m", p=128))
        load_xt(1)
        load_xt(2)
        load_xt(3)
        nc.vector.memset(ones1, 1.0)
        # preload the exp table set off the critical path
        nc.scalar.activation(out=warm, in_=ones1, func=EXPF, scale=1.0)
        kk8 = sb.tile([128, KT, 128], E4M3)      # [K8^T; resid] per key tile
        vn = sb.tile([128, KT, D + 1], BF16)     # V rows + ones column
        nc.vector.memset(vn, 1.0)
        aout = sb.tile([128, 2, NSEQ], BF16)     # attention out (normalized)

        ps_s = ctx.enter_context(tc.tile_pool(name="ps_s", bufs=2, space="PSUM"))
        ps_pv = ctx.enter_context(tc.tile_pool(name="ps_pv", bufs=4, space="PSUM"))
        q8tp = ctx.enter_context(tc.tile_pool(name="q8tp", bufs=2))
        q8ap = ctx.enter_context(tc.tile_pool(name="q8ap", bufs=2))
        ptp = ctx.enter_context(tc.tile_pool(name="ptp", bufs=14))
        rrp = ctx.enter_context(tc.tile_pool(name="rrp", bufs=2))
        Rp_pool = ctx.enter_context(tc.tile_pool(name="Rp", bufs=4))
        outp = ctx.enter_context(tc.tile_pool(name="outp", bufs=3))
        vntp = ctx.enter_context(tc.tile_pool(name="vntp", bufs=2))

        def strided_ap(t, offset_elems, dims):
            return bass.AP(tensor=t.tensor, offset=t.offset + offset_elems, ap=dims)

        vTsb = sb.tile([64, NSEQ], BF16)

        def emit_sgroup(sg):
            """Load x^T chunk sg via DMA transpose; project K (dual-e4m3) and V."""
            # K^T -> pk rows 0:64; V^T -> pk rows 64:128 (shared psum tile)
            pk = ps_s.tile([128, 1024], F32, tag="sc", name=f"pk{sg}")
            for d in range(DKT):
                nc.tensor.matmul(pk[0:64, 0:512], wkv_sb[:, d, 0:D], xT[:, d, ds(sg * 512, 512)],
                                 start=(d == 0), stop=(d == DKT - 1))
            for d in range(DKT):
                nc.tensor.matmul(pk[ds(64, 64), 0:512], wkv_sb[:, d, ds(D, D)], xT[:, d, ds(sg * 512, 512)],
                                 start=(d == 0), stop=(d == DKT - 1))
            # kk8 rows 0:64 = e4m3(K^T); rows 64:128 = e4m3(K^T - K8)
            k_dst = kk8[0:64, ds(sg * 4, 4), :]
            nc.vector.tensor_copy(k_dst, pk[0:64, 0:512])
            nc.vector.tensor_sub(kk8[ds(64, 64), ds(sg * 4, 4), :], pk[0:64, 0:512], k_dst)
            # V^T to sbuf bf16, xbar-transpose to a contiguous scratch (the xbar
            # writer mislays tiles into strided dests), then copy into vn rows
            nc.scalar.copy(vTsb[:, ds(sg * 512, 512)], pk[ds(64, 64), 0:512])
            if sg == 0:
                # wo load rides the Act queue behind the first V copy, keeping
                # HWDGE free for the prologue transposes
                nc.scalar.dma_start(out=wo_sb, in_=wo.rearrange("(t p) m -> p t m", p=128))
            vnt = vntp.tile([128, 4, D], BF16, tag="vnt", name=f"vnt{sg}")
            nc.sync.dma_start_transpose(vnt, vTsb[:, ds(sg * 512, 512)])
            nc.scalar.copy(vn[:, ds(sg * 4, 4), 0:D], vnt)

        def emit_qt(qc):
            """Q projection for all 4 heads + dual-e4m3 prep + dup-spread DMAs."""
            q8t = q8tp.tile([128, 2, 2, 512], E4M3, tag="q8t", name=f"q8t{qc}")
            for p in range(2):
                pq = ps_s.tile([128, 1024], F32, tag="sc", name=f"pq{qc}_{p}")
                for d in range(DKT):
                    nc.tensor.matmul(pq[:, 0:512], wq_sb[:, d, ts(p, 128)], xT[:, d, ds(qc * 512, 512)],
                                     start=(d == 0), stop=(d == DKT - 1))
                nc.vector.tensor_copy(q8t[:, p, 0, :], pq[:, 0:512])
                nc.vector.tensor_sub(q8t[:, p, 1, :], pq[:, 0:512], q8t[:, p, 0, :])
            q8a = q8ap.tile([128, 4, 2, 512], E4M3, tag="q8a", name=f"q8a{qc}")
            dup_eng = nc.scalar if qc == 0 else nc.sync
            # spread/dup: head h = 2p + e lives at q8a[:, h, :, :] with rows 0:64 == 64:128
            for e in range(2):
                src = strided_ap(q8t, 0, [[q8t.ap[0][0], 64], [1024, 2], [1, 1024]])
                if e == 1:
                    src = strided_ap(q8t, 64 * q8t.ap[0][0],
                                     [[q8t.ap[0][0], 64], [1024, 2], [1, 1024]])
                for half in range(2):
                    dst = strided_ap(q8a, 64 * half * q8a.ap[0][0] + e * 1024,
                                     [[q8a.ap[0][0], 64], [2048, 2], [1, 1024]])
                    dup_eng.dma_start(out=dst, in_=src)
            return q8a

        state = {}
        q8_cur = {}

        norm_rtb = {}

        def emit_norm_recip(qc):
            pvs = state[qc]
            rr = rrp.tile([128, 2048], F32, tag="rr")
            Rtb = Rp_pool.tile([128, 2048], F32, tag="R")
            norm_rtb[qc] = Rtb
            for h in range(4):
                # reciprocal lands on partition 0 (partition_broadcast reads
                # physical partition 0 only)
                nc.vector.reciprocal(out=rr[ds(0, 1), ds(h * 512, 512)], in_=pvs[h][ds(64, 1), :])
                nc.gpsimd.partition_broadcast(Rtb[:, ds(h * 512, 512)], rr[ds(0, 1), ds(h * 512, 512)])

        def emit_norm_mul(qc):
            pvs = state[qc]
            Rtb = norm_rtb.pop(qc)
            for p in range(2):
                for i in range(2):
                    h = 2 * p + i
                    nc.vector.tensor_mul(aout[ds(i * 64, 64), p, ds(qc * 512, 512)],
                                         pvs[h][0:64, :], Rtb[ds(i * 64, 64), ds(h * 512, 512)])

        def emit_outproj(qc, ops=None):
            for op_ in (range(DIM // 256) if ops is None else ops):
                po = ps_s.tile([128, 1024], F32, tag="sc", name=f"po{qc}_{op_}")
                for e in range(2):
                    od = 2 * op_ + e
                    nc.tensor.matmul(po[:, ds(e * 512, 512)], wo_sb[:, 0, ts(od, 128)],
                                     aout[:, 0, ds(qc * 512, 512)], start=True, stop=False)
                    nc.tensor.matmul(po[:, ds(e * 512, 512)], wo_sb[:, 1, ts(od, 128)],
                                     aout[:, 1, ds(qc * 512, 512)], start=False, stop=True)
                ot = outp.tile([128, 1024], BF16, tag="ot")
                nc.scalar.copy(ot, po)
                dst = bass.AP(tensor=out.tensor,
                              offset=out.offset + (2 * op_ * 128) * NSEQ + qc * 512,
                              ap=[[NSEQ, 128], [128 * NSEQ, 2], [1, 512]])
                src = strided_ap(ot, 0, [[ot.ap[0][0], 128], [512, 2], [1, 512]])
                nc.scalar.dma_start(out=dst, in_=src)

        pending_pv = []

        def flush_group(group):
            # t-outer / head-inner so consecutive matmuls share the vn[kt]
            # stationary tile.
            for t in range(2):
                for (qc_, j_, h_, pt_) in group:
                    kt = 2 * j_ + t
                    nc.tensor.matmul(state[qc_][h_][0:65, :],
                                     vn[:, kt, :], pt_[:, ds(t * 512, 512)],
                                     start=(kt == 0), stop=(kt == KT - 1))

        def flush_pv():
            while pending_pv:
                flush_group(pending_pv[:4])
                del pending_pv[:4]

        def emit_quanta(qc, j):
            q8a = q8_cur[qc]
            new_pv = []
            for h in range(4):
                psc = ps_s.tile([128, 1024], F32, tag="sc", name=f"psc{qc}_{j}_{h}")
                for t in range(2):
                    kt = 2 * j + t
                    kk = kk8[:, kt, :]
                    lhsT = bass.AP(tensor=kk.tensor, offset=kk.offset,
                                   ap=[[kk.ap[0][0], 128], [0, 2], [1, 128]])
                    nc.tensor.matmul(psc[:, ts(t, 512)], lhsT, q8a[:, h, :, :],
                                     start=True, stop=True, perf_mode=DR)
                pt = ptp.tile([128, 1024], BF16, tag="pt", name=f"pt{qc}_{j}_{h}")
                # DVE exps only mid-qc (j 2..6) so boundary norm work (recip,
                # muls on DVE) never delays the psc rotation; Act covers j0,1,7
                dve_exp = (h >= 2 and 2 <= j <= 5) or (h >= 2 and j == 6) \
                    or (h == 1 and j in (3, 4))
                use_act = not dve_exp
                if use_act:
                    nc.scalar.activation(out=pt, in_=psc, func=EXPF, scale=SCALE)
                else:
                    nc.vector.tensor_scalar(pt.bitcast(I16), psc, A_SCH, C_SCH, MULT, ADD)
                new_pv.append((qc, j, h, pt))
            # keep up to three quanta of exp tiles in flight before P@V
            if len(pending_pv) >= 12:
                flush_group(pending_pv[:4])
                del pending_pv[:4]
            pending_pv.extend(new_pv)

        # interleaved prologue: per key chunk, project K/V then run qc=0 attention on it
        state[0] = [ps_pv.tile([128, 512], F32, tag="pv", name=f"pv0_{h}") for h in range(4)]
        q8_cur[0] = emit_qt(0)
        for sg in range(QC):
            emit_sgroup(sg)
            emit_quanta(0, 2 * sg)
            emit_quanta(0, 2 * sg + 1)
            if sg == 2:
                q8_cur[1] = emit_qt(1)
        flush_pv()
        emit_norm_recip(0)
        for qc in range(1, QC):
            pvs = [ps_pv.tile([128, 512], F32, tag="pv", name=f"pv{qc}_{h}") for h in range(4)]
            state[qc] = pvs
            for j in range(KT // 2):
                emit_quanta(qc, j)
                if j == 1:
                    emit_norm_mul(qc - 1)
                if 2 <= j <= 5:
                    emit_outproj(qc - 1, ops=[j - 2])
                if j == 4 and qc + 1 < QC:
                    q8_cur[qc + 1] = emit_qt(qc + 1)
            flush_pv()
            emit_norm_recip(qc)
        emit_norm_mul(QC - 1)
        emit_outproj(QC - 1, ops=[0, 1])
        emit_outproj(QC - 1, ops=[2, 3])

    nc.compile()
    return nc


_CACHE = {}


def _get_nc(NSEQ):
    if NSEQ not in _CACHE:
        _CACHE[NSEQ] = build_nc(NSEQ)
    return _CACHE[NSEQ]


def kernel(x, Wq, Wk, Wv, Wo, bo):
    """Full-input entry point: shard over 8 cores, run, gather."""
    import ml_dtypes
    bf16 = ml_dtypes.bfloat16
    x = np.asarray(x, np.float32)
    Wq, Wk, Wv, Wo, bo = (np.asarray(a, np.float32) for a in (Wq, Wk, Wv, Wo, bo))
    B, N, C = x.shape
    nc = _get_nc(N)
    in_maps = []
    for c in range(8):
        b, g = c // 4, c % 4
        in_maps.append({
            "xt": np.ascontiguousarray(x[b].T).astype(bf16),
            "wq": np.ascontiguousarray(Wq[:, g * 256:(g + 1) * 256]).astype(bf16),
            "wkv": np.ascontiguousarray(
                np.concatenate([Wk[:, g * D:(g + 1) * D], Wv[:, g * D:(g + 1) * D]], axis=1)
            ).astype(bf16),
            "wo": np.ascontiguousarray(Wo[g * 256:(g + 1) * 256, :]).astype(bf16),
        })
    res = bass_utils.run_bass_kernel_spmd(nc, in_maps, core_ids=list(range(8)))
    outs = [res.results[c]["out"] for c in range(8)]
    full = np.empty((B, N, C), np.float32)
    for b in range(B):
        acc = outs[4 * b].astype(np.float32)
        for g in range(1, 4):
            acc = acc + outs[4 * b + g]
        full[b] = acc.T + bo[None, :]
    return full
